# revision 1
# baseline (speedup 1.0000x reference)
import sys, functools

if "/opt/trn_rl_repo" not in sys.path:
    sys.path.insert(0, "/opt/trn_rl_repo")

import numpy as np
import ml_dtypes

from concourse import bacc
import concourse.bass as bass
import concourse.mybir as mybir
import concourse.tile as tile
from concourse.bass_utils import run_bass_kernel_spmd
from concourse.masks import make_identity

BF16 = mybir.dt.bfloat16
F32 = mybir.dt.float32
AF = mybir.ActivationFunctionType
ALU = mybir.AluOpType
AX = mybir.AxisListType

S, D, HD, H, MLPH = 4096, 3072, 128, 24, 9216
NCORES = 8
HL = H // NCORES            # 3 heads per core
FQ = HL * HD                # 384
FM = MLPH // NCORES         # 1152
FAB = 2 * FM                # 2304 (a/b interleaved in 128-col pairs)
FQKV = 3 * FQ               # 1152
FTOT = FQKV + FAB           # 3456
NCOL = D // NCORES          # 384 output cols per core
EPS = 1e-6
SCH = 1024                  # s-chunk for projection phase
NSC = S // SCH              # 4
KT = D // 128               # 24 contraction tiles

LAST_RESULT = None          # test.py introspection


def _to_bf16(a):
    """Fast round-to-nearest f32 -> bf16."""
    a = np.ascontiguousarray(a, np.float32)
    u = a.view(np.uint32)
    r = ((u >> 16) & 1) + np.uint32(0x7FFF)
    return ((u + r) >> 16).astype(np.uint16).view(ml_dtypes.bfloat16)


@functools.lru_cache(maxsize=1)
def _build():
    nc = bacc.Bacc(
        "TRN2",
        target_bir_lowering=False,
        debug=False,
        enable_asserts=False,
        num_devices=NCORES,
    )
    x = nc.dram_tensor("x", [S, D], F32, kind="ExternalInput").ap()
    w1q = nc.dram_tensor("w1qkv", [D, FQKV], BF16, kind="ExternalInput").ap()
    w1ab = nc.dram_tensor("w1ab", [D, FAB], BF16, kind="ExternalInput").ap()
    c2 = nc.dram_tensor("c2", [1, FTOT], BF16, kind="ExternalInput").ap()
    cosb = nc.dram_tensor("cosb", [S, HD], BF16, kind="ExternalInput").ap()
    sinb = nc.dram_tensor("sinb", [S, HD], BF16, kind="ExternalInput").ap()
    qwb = nc.dram_tensor("qwb", [128, HD], F32, kind="ExternalInput").ap()
    kwb = nc.dram_tensor("kwb", [128, HD], F32, kind="ExternalInput").ap()
    wout = nc.dram_tensor("wout", [D + MLPH, NCOL], BF16, kind="ExternalInput").ap()
    res = nc.dram_tensor("res", [S, NCOL], F32, kind="ExternalInput").ap()
    out = nc.dram_tensor("out", [S, NCOL], F32, kind="ExternalOutput").ap()

    rg = [list(range(NCORES))]

    with tile.TileContext(nc) as tc:
        with (
            tc.tile_pool(name="const", bufs=1) as const,
            tc.tile_pool(name="dram", bufs=1, space="DRAM") as dram,
        ):
            ident = const.tile([128, 128], BF16)
            make_identity(nc, ident)
            ones_s = const.tile([1, 512], BF16)
            nc.vector.memset(ones_s, 1.0)
            ones_p = const.tile([1, 128], BF16)
            nc.vector.memset(ones_p, 1.0)
            eps_sb = const.tile([128, 1], F32)
            nc.vector.memset(eps_sb, EPS)
            qwb_sb = const.tile([128, HD], F32)
            nc.sync.dma_start(qwb_sb, qwb)
            kwb_sb = const.tile([128, HD], F32)
            nc.sync.dma_start(kwb_sb, kwb)
            c2_sb = const.tile([1, FTOT], BF16)
            nc.sync.dma_start(c2_sb, c2)
            # resident QKV weight: [dm_part, kt, 1152]
            w1q_sb = const.tile([128, KT, FQKV], BF16)
            nc.sync.dma_start(w1q_sb, w1q.rearrange("(kt p) f -> p kt f", p=128))

            qr_d = dram.tile([S, FQ], BF16)
            kr_d = dram.tile([S, FQ], BF16)
            v_d = dram.tile([S, FQ], BF16)
            mlp_b = dram.tile([FM, S], BF16)
            mlp_g = dram.tile([MLPH, S], BF16, addr_space="Shared")
            att_b = dram.tile([FQ, S], BF16)
            att_g = dram.tile([D, S], BF16, addr_space="Shared")

            # ---------------- Phase P: LN + transpose + QKV/MLP projection ----
            with (
                tc.tile_pool(name="xp", bufs=2) as xp,
                tc.tile_pool(name="tp", bufs=3) as tp,
                tc.tile_pool(name="ttp", bufs=1) as ttp,
                tc.tile_pool(name="w1s", bufs=6) as w1s,
                tc.tile_pool(name="smal", bufs=12) as smal,
                tc.tile_pool(name="stg", bufs=3) as stg,
                tc.tile_pool(name="abp", bufs=3) as abp,
                tc.tile_pool(name="psab", bufs=1, space="PSUM") as psab,
                tc.tile_pool(name="pst", bufs=2, space="PSUM") as pst,
                tc.tile_pool(name="psq", bufs=2, space="PSUM") as psq,
            ):
                for sc in range(NSC):
                    tT = ttp.tile([128, KT, SCH], BF16, tag="tT")
                    for ss in range(8):
                        s0 = sc * SCH + ss * 128
                        xt = xp.tile([128, D], F32, tag="x")
                        nc.sync.dma_start(xt, x[s0 : s0 + 128, :])
                        s1 = smal.tile([128, 1], F32, tag="s1")
                        nc.vector.reduce_sum(s1, xt, axis=AX.X)
                        nmu = smal.tile([128, 1], F32, tag="nmu")
                        nc.scalar.mul(nmu, s1, -1.0 / D)
                        sqs = tp.tile([128, D], BF16, tag="sq", bufs=2)
                        v2 = smal.tile([128, 1], F32, tag="v2")
                        nc.scalar.activation(sqs, xt, AF.Square, bias=nmu, scale=1.0,
                                             accum_out=v2)
                        std = smal.tile([128, 1], F32, tag="std")
                        nc.scalar.activation(std, v2, AF.Sqrt, bias=eps_sb, scale=1.0 / D)
                        rstd = smal.tile([128, 1], F32, tag="rstd")
                        nc.vector.reciprocal(rstd, std)
                        nmr = smal.tile([128, 1], F32, tag="nmr")
                        nc.vector.tensor_mul(out=nmr, in0=nmu, in1=rstd)
                        tti = tp.tile([128, D], BF16, tag="t")
                        nc.scalar.activation(tti, xt, AF.Identity, bias=nmr, scale=rstd)
                        # transpose 24 [128,128] blocks -> tT, grouped 4-per-psum
                        for kg in range(KT // 4):
                            ptt = pst.tile([128, 4, 128], BF16, tag="ptt")
                            for k4 in range(4):
                                ktd = kg * 4 + k4
                                nc.tensor.transpose(
                                    ptt[:, k4, :],
                                    tti[:, ktd * 128 : (ktd + 1) * 128], ident)
                            nc.any.tensor_copy(
                                out=tT[:, kg * 4 : kg * 4 + 4, ss * 128 : (ss + 1) * 128],
                                in_=ptt)

                    # --- QKV pass (s-major): one psum bank per j ---
                    for ss in range(8):
                        s0 = sc * SCH + ss * 128
                        sl = ss * 128
                        cos_t = stg.tile([128, HD], BF16, tag="cos")
                        nc.sync.dma_start(cos_t, cosb[s0 : s0 + 128, :])
                        sin_t = stg.tile([128, HD], BF16, tag="sin")
                        nc.sync.dma_start(sin_t, sinb[s0 : s0 + 128, :])
                        c2_ = cos_t.rearrange("p (x two) -> p x two", two=2)
                        s2_ = sin_t.rearrange("p (x two) -> p x two", two=2)
                        for j in range(3):
                            pq = psq.tile([128, 512], F32, tag="pqkv")
                            for kt in range(KT):
                                nc.tensor.matmul(
                                    pq[:, :FQ],
                                    tT[:, kt, sl : sl + 128],
                                    w1q_sb[:, kt, j * FQ : (j + 1) * FQ],
                                    start=(kt == 0), stop=False)
                            nc.tensor.matmul(
                                pq[:, :FQ], ones_p,
                                c2_sb[:, j * FQ : (j + 1) * FQ],
                                start=False, stop=True)
                            if j == 2:
                                vstg = stg.tile([128, FQ], BF16, tag="vst", bufs=2)
                                nc.scalar.copy(vstg, pq[:, :FQ])
                                nc.sync.dma_start(v_d[s0 : s0 + 128, :], vstg)
                                continue
                            wb = qwb_sb if j == 0 else kwb_sb
                            dst = qr_d if j == 0 else kr_d
                            qn = stg.tile([128, FQ], BF16, tag=f"qn{j}")
                            qrr = stg.tile([128, FQ], BF16, tag=f"qr{j}")
                            tmp = stg.tile([128, FQ], BF16, tag=f"tm{j}")
                            for hh in range(HL):
                                blk = pq[:, hh * HD : (hh + 1) * HD]
                                ssq = smal.tile([128, 1], F32, tag="ssq")
                                sq2 = stg.tile([128, HD], F32, tag="sq2")
                                nc.scalar.activation(sq2, blk, AF.Square, accum_out=ssq)
                                sstd = smal.tile([128, 1], F32, tag="sstd")
                                nc.scalar.activation(sstd, ssq, AF.Sqrt,
                                                     bias=eps_sb, scale=1.0 / HD)
                                rst = smal.tile([128, 1], F32, tag="rst")
                                nc.vector.reciprocal(rst, sstd)
                                qnb = qn[:, hh * HD : (hh + 1) * HD]
                                nc.vector.scalar_tensor_tensor(
                                    qnb, blk, rst, wb, ALU.mult, ALU.mult)
                                q3 = qnb.rearrange("p (x two) -> p x two", two=2)
                                t3 = tmp[:, hh * HD : (hh + 1) * HD].rearrange(
                                    "p (x two) -> p x two", two=2)
                                nc.vector.tensor_mul(out=t3[:, :, 0], in0=q3[:, :, 1],
                                                     in1=s2_[:, :, 0])
                                nc.vector.tensor_mul(out=t3[:, :, 1], in0=q3[:, :, 0],
                                                     in1=s2_[:, :, 1])
                                nc.vector.tensor_mul(
                                    out=qrr[:, hh * HD : (hh + 1) * HD],
                                    in0=qnb, in1=cos_t)
                            nc.vector.tensor_add(out=qrr, in0=qrr, in1=tmp)
                            nc.sync.dma_start(dst[s0 : s0 + 128, :], qrr)

                    # --- a/b (f-major) + SwiGLU ---
                    for fb in range(9):
                        pa = psab.tile([128, 2, 2, 512], F32, tag="pab")
                        for kt in range(KT):
                            wt = w1s.tile([128, 256], BF16, tag="w1ab")
                            nc.sync.dma_start(
                                wt, w1ab[kt * 128 : (kt + 1) * 128,
                                         fb * 256 : (fb + 1) * 256])
                            for f2 in range(2):
                                for sh in range(2):
                                    nc.tensor.matmul(
                                        pa[:, f2, sh, :],
                                        wt[:, f2 * 128 : (f2 + 1) * 128],
                                        tT[:, kt, sh * 512 : (sh + 1) * 512],
                                        start=(kt == 0), stop=False)
                        for f2 in range(2):
                            c0 = FQKV + fb * 256 + f2 * 128
                            for sh in range(2):
                                nc.tensor.matmul(
                                    pa[:, f2, sh, :],
                                    c2_sb[:, c0 : c0 + 128], ones_s,
                                    start=False, stop=True)
                        a_sb = abp.tile([128, 2, 512], BF16, tag="asb")
                        nc.scalar.activation(a_sb, pa[:, 0], AF.Silu)
                        m_sb = abp.tile([128, 2, 512], BF16, tag="msb")
                        nc.vector.tensor_mul(out=m_sb, in0=a_sb, in1=pa[:, 1])
                        nc.sync.dma_start(
                            mlp_b[fb * 128 : (fb + 1) * 128,
                                  sc * SCH : (sc + 1) * SCH],
                            m_sb.rearrange("p a b -> p (a b)"))

            nc.gpsimd.collective_compute(
                "AllGather", ALU.bypass, replica_groups=rg,
                ins=[mlp_b.opt()], outs=[mlp_g.opt()])

            # ---------------- Phase A: attention ------------------------------
            with (
                tc.tile_pool(name="attq", bufs=2) as attq,
                tc.tile_pool(name="attv", bufs=2) as attv,
                tc.tile_pool(name="attp", bufs=5) as attp,
                tc.tile_pool(name="attP", bufs=2) as attP,
                tc.tile_pool(name="atts", bufs=4) as atts,
                tc.tile_pool(name="psS", bufs=3, space="PSUM") as psS,
                tc.tile_pool(name="psT", bufs=2, space="PSUM") as psT,
                tc.tile_pool(name="psV", bufs=2, space="PSUM") as psV,
            ):
                for h in range(HL):
                    qT = attq.tile([128, S], BF16, tag="qT")
                    kT = attq.tile([128, S], BF16, tag="kT")
                    for srcd, dstT in ((qr_d, qT), (kr_d, kT)):
                        for sg in range(8):  # groups of 4 s-tiles
                            ptt = psT.tile([128, 4, 128], BF16, tag="ptp")
                            for s4 in range(4):
                                st = sg * 4 + s4
                                qs = atts.tile([128, HD], BF16, tag="qs")
                                nc.sync.dma_start(
                                    qs, srcd[st * 128 : (st + 1) * 128,
                                             h * HD : (h + 1) * HD])
                                nc.tensor.transpose(ptt[:, s4, :], qs, ident)
                            nc.any.tensor_copy(
                                out=dstT[:, sg * 512 : (sg + 1) * 512],
                                in_=ptt.rearrange("p a b -> p (a b)"))
                    v_sb = attv.tile([128, 32, HD], BF16, tag="vsb")
                    nc.sync.dma_start(
                        v_sb, v_d.rearrange("(t p) f -> p t f", p=128)[
                            :, :, h * HD : (h + 1) * HD])
                    for sqc in range(8):
                        pn_tiles = []
                        for sqs in range(4):
                            sq0 = sqc * 512 + sqs * 128
                            dacc = atts.tile([128, 8], F32, tag="dacc")
                            P = attP.tile([128, S], BF16, tag="P")
                            for skc in range(8):
                                pss = psS.tile([128, 512], F32, tag="pss")
                                nc.tensor.matmul(
                                    pss, qT[:, sq0 : sq0 + 128],
                                    kT[:, skc * 512 : (skc + 1) * 512],
                                    start=True, stop=True)
                                nc.scalar.activation(
                                    P[:, skc * 512 : (skc + 1) * 512], pss, AF.Exp,
                                    accum_out=dacc[:, skc : skc + 1])
                            den = atts.tile([128, 1], F32, tag="den")
                            nc.vector.reduce_sum(den, dacc, axis=AX.X)
                            invd = atts.tile([128, 1], F32, tag="invd")
                            nc.vector.reciprocal(invd, den)
                            Pn = attp.tile([128, S], BF16, tag="Pn")
                            nc.scalar.activation(Pn, P, AF.Identity, scale=invd)
                            pn_tiles.append(Pn)
                        pacc = psV.tile([128, 512], F32, tag="pacc")
                        for sk in range(32):
                            ptp = psT.tile([128, 4, 128], BF16, tag="ptp")
                            for sqs in range(4):
                                nc.tensor.transpose(
                                    ptp[:, sqs, :],
                                    pn_tiles[sqs][:, sk * 128 : (sk + 1) * 128],
                                    ident)
                            PT = atts.tile([128, 512], BF16, tag="PT")
                            nc.any.tensor_copy(
                                out=PT, in_=ptp.rearrange("p a b -> p (a b)"))
                            nc.tensor.matmul(pacc, v_sb[:, sk, :], PT,
                                             start=(sk == 0), stop=(sk == 31))
                        att_o = atts.tile([128, 512], BF16, tag="atto")
                        nc.scalar.copy(att_o, pacc)
                        nc.sync.dma_start(
                            att_b[h * HD : (h + 1) * HD,
                                  sqc * 512 : (sqc + 1) * 512], att_o)

            nc.gpsimd.collective_compute(
                "AllGather", ALU.bypass, replica_groups=rg,
                ins=[att_b.opt()], outs=[att_g.opt()])

            # ---------------- Phase O: output projection + epilogue -----------
            with (
                tc.tile_pool(name="wo", bufs=1) as wo,
                tc.tile_pool(name="lop", bufs=8) as lop,
                tc.tile_pool(name="eop", bufs=4) as eop,
                tc.tile_pool(name="psO", bufs=2, space="PSUM") as psO,
            ):
                wo_sb = wo.tile([128, 96, NCOL], BF16)
                nc.sync.dma_start(wo_sb, wout.rearrange("(kt p) n -> p kt n", p=128))
                for sb in range(8):
                    po = psO.tile([128, 4, 512], F32, tag="po")
                    for kt in range(96):
                        lt = lop.tile([128, 512], BF16, tag="lhsT")
                        if kt < KT:
                            src = att_g[kt * 128 : (kt + 1) * 128,
                                        sb * 512 : (sb + 1) * 512]
                        else:
                            src = mlp_g[(kt - KT) * 128 : (kt - KT + 1) * 128,
                                        sb * 512 : (sb + 1) * 512]
                        nc.sync.dma_start(lt, src)
                        for st in range(4):
                            nc.tensor.matmul(
                                po[:, st, :NCOL],
                                lt[:, st * 128 : (st + 1) * 128],
                                wo_sb[:, kt, :],
                                start=(kt == 0), stop=(kt == 95))
                    for st in range(4):
                        s0 = sb * 512 + st * 128
                        rt = eop.tile([128, NCOL], F32, tag="rest")
                        nc.sync.dma_start(rt, res[s0 : s0 + 128, :])
                        ot = eop.tile([128, NCOL], F32, tag="ot")
                        nc.vector.tensor_add(out=ot, in0=po[:, st, :NCOL], in1=rt)
                        nc.sync.dma_start(out[s0 : s0 + 128, :], ot)

    nc.finalize()
    return nc


def _prep(inputs):
    hs = np.asarray(inputs["hidden_states"], np.float32).reshape(S, D)
    temb = np.asarray(inputs["temb_mod"], np.float32).reshape(3 * D)
    shift, scale, gate = temb[:D], temb[D : 2 * D], temb[2 * D :]
    cos = np.asarray(inputs["rotary_cos"], np.float32)
    sin = np.asarray(inputs["rotary_sin"], np.float32)
    w1 = np.asarray(inputs["w_qkv_mlp"], np.float32)
    wa = np.asarray(inputs["w_out_attn"], np.float32)
    wm = np.asarray(inputs["w_out_mlp"], np.float32)
    nqw = np.asarray(inputs["norm_q_w"], np.float32)
    nkw = np.asarray(inputs["norm_k_w"], np.float32)

    sgn = np.ones(HD, np.float32)
    sgn[0::2] = -1.0
    cosb = _to_bf16(cos)
    sinb = _to_bf16(sin * sgn)
    alpha = float(HD) ** -0.25
    qwb = np.tile((nqw * alpha)[None, :], (128, 1)).astype(np.float32)
    kwb = np.tile((nkw * alpha)[None, :], (128, 1)).astype(np.float32)
    onep = (1.0 + scale)[:, None]

    in_maps = []
    for c in range(NCORES):
        q0, k0, v0 = c * FQ, D + c * FQ, 2 * D + c * FQ
        a0, b0 = 3 * D + c * FM, 3 * D + MLPH + c * FM
        w1qkv = np.concatenate(
            [w1[:, q0 : q0 + FQ], w1[:, k0 : k0 + FQ], w1[:, v0 : v0 + FQ]], axis=1)
        a_c = w1[:, a0 : a0 + FM].reshape(D, 9, 128)
        b_c = w1[:, b0 : b0 + FM].reshape(D, 9, 128)
        w1ab_c = np.stack([a_c, b_c], axis=2).reshape(D, FAB)
        c2_c = np.concatenate([shift @ w1qkv, shift @ w1ab_c])[None, :]
        n0 = c * NCOL
        g = gate[n0 : n0 + NCOL][None, :]
        wout_c = np.concatenate(
            [wa[:, n0 : n0 + NCOL] * g, wm[:, n0 : n0 + NCOL] * g], axis=0)
        in_maps.append(dict(
            x=hs,
            w1qkv=_to_bf16(w1qkv * onep),
            w1ab=_to_bf16(w1ab_c * onep),
            c2=_to_bf16(c2_c),
            cosb=cosb, sinb=sinb, qwb=qwb, kwb=kwb,
            wout=_to_bf16(wout_c),
            res=np.ascontiguousarray(hs[:, n0 : n0 + NCOL]),
        ))
    return in_maps


def kernel(**inputs):
    global LAST_RESULT
    nc = _build()
    in_maps = _prep(inputs)
    r = run_bass_kernel_spmd(nc, in_maps, core_ids=list(range(NCORES)))
    LAST_RESULT = r
    full = np.concatenate([m["out"] for m in r.results], axis=1)
    return full.reshape(1, S, D).astype(np.float32)



# revision 3
# speedup vs baseline: 1.2596x; 1.2596x over previous
import sys, functools

if "/opt/trn_rl_repo" not in sys.path:
    sys.path.insert(0, "/opt/trn_rl_repo")

import numpy as np
import ml_dtypes

from concourse import bacc
import concourse.bass as bass
import concourse.mybir as mybir
import concourse.tile as tile
from concourse.bass_utils import run_bass_kernel_spmd
from concourse.masks import make_identity

BF16 = mybir.dt.bfloat16
F32 = mybir.dt.float32
AF = mybir.ActivationFunctionType
ALU = mybir.AluOpType
AX = mybir.AxisListType

S, D, HD, H, MLPH = 4096, 3072, 128, 24, 9216
NCORES = 8
HL = H // NCORES            # 3 heads per core
FQ = HL * HD                # 384
FM = MLPH // NCORES         # 1152
FMT = FM // 128             # 9 f-tiles of mlp hidden
FAB = 2 * FM                # 2304 (a/b interleaved in 128-col pairs)
FQKV = 3 * FQ               # 1152
NCOL = D // NCORES          # 384 output cols per core
FO = FQ + FM                # 1536 rows of fused output weight
WOT = FO // 128             # 12 contraction tiles of output proj
EPS = 1e-6
SCH = 1024                  # s-chunk for projection phase
NSC = S // SCH              # 4
KT = D // 128               # 24 contraction tiles of input proj
NKT = S // 128              # 32 k tiles in attention
QC = 512                    # q-chunk for attention/output phase
NQC = S // QC               # 8
NRS = 4                     # number of ReduceScatter chunks
RSW = S // NRS              # 1024 s-columns per RS chunk
SUBRS = NQC // NRS          # q-chunks per RS chunk

LAST_RESULT = None          # test.py introspection


def _to_bf16(a):
    """Fast round-to-nearest f32 -> bf16."""
    a = np.ascontiguousarray(a, np.float32)
    u = a.view(np.uint32)
    r = ((u >> 16) & 1) + np.uint32(0x7FFF)
    return ((u + r) >> 16).astype(np.uint16).view(ml_dtypes.bfloat16)


@functools.lru_cache(maxsize=1)
def _build():
    nc = bacc.Bacc(
        "TRN2",
        target_bir_lowering=False,
        debug=False,
        enable_asserts=False,
        num_devices=NCORES,
    )
    x = nc.dram_tensor("x", [S, D], BF16, kind="ExternalInput").ap()
    w1q = nc.dram_tensor("w1qkv", [D, FQKV], BF16, kind="ExternalInput").ap()
    w1ab = nc.dram_tensor("w1ab", [D, FAB], BF16, kind="ExternalInput").ap()
    c2q = nc.dram_tensor("c2q", [1, FQKV], BF16, kind="ExternalInput").ap()
    c2ab = nc.dram_tensor("c2ab", [128, 2 * FMT], F32, kind="ExternalInput").ap()
    cosb = nc.dram_tensor("cosb", [S, HD], BF16, kind="ExternalInput").ap()
    sinb = nc.dram_tensor("sinb", [S, HD], BF16, kind="ExternalInput").ap()
    qwb = nc.dram_tensor("qwb", [128, HD], F32, kind="ExternalInput").ap()
    kwb = nc.dram_tensor("kwb", [128, HD], F32, kind="ExternalInput").ap()
    wout = nc.dram_tensor("wout", [FO, D], BF16, kind="ExternalInput").ap()
    resT = nc.dram_tensor("resT", [NCOL, S], F32, kind="ExternalInput").ap()
    out_t = nc.dram_tensor("out", [NCOL, S], F32, kind="ExternalOutput").ap()

    rg = [list(range(NCORES))]

    with tile.TileContext(nc) as tc:
        with (
            tc.tile_pool(name="const", bufs=1) as const,
            tc.tile_pool(name="dram", bufs=1, space="DRAM") as dram,
        ):
            ident = const.tile([128, 128], BF16)
            make_identity(nc, ident)
            ones_p = const.tile([1, 128], BF16)
            nc.vector.memset(ones_p, 1.0)
            ones128 = const.tile([128, 128], BF16)
            nc.vector.memset(ones128, 1.0)
            eps_sb = const.tile([128, 1], F32)
            nc.vector.memset(eps_sb, EPS)
            qwb_sb = const.tile([128, HD], F32)
            nc.sync.dma_start(qwb_sb, qwb)
            kwb_sb = const.tile([128, HD], F32)
            nc.sync.dma_start(kwb_sb, kwb)
            c2q_sb = const.tile([1, FQKV], BF16)
            nc.sync.dma_start(c2q_sb, c2q)
            c2ab_sb = const.tile([128, 2 * FMT], F32)
            nc.sync.dma_start(c2ab_sb, c2ab)

            qT_d = dram.tile([FQ, S], BF16)     # rope(q) transposed, d-major
            kT_d = dram.tile([FQ, S], BF16)
            v_d = dram.tile([S, FQ], BF16)
            m_f = dram.tile([FM, S], BF16)      # swiglu output, f-major
            pall = [dram.tile([D, RSW], BF16, tag=f"pall{i}", name=f"pall{i}")
                    for i in range(NRS)]
            rs_out = [dram.tile([NCOL, RSW], BF16, tag=f"rso{i}", name=f"rso{i}")
                      for i in range(NRS)]

            # ---------------- Phase P: LN + transpose + QKV/MLP projection ----
            with (
                tc.tile_pool(name="w1qp", bufs=1) as w1qp,
                tc.tile_pool(name="xp", bufs=2) as xp,
                tc.tile_pool(name="tp", bufs=3) as tp,
                tc.tile_pool(name="ttp", bufs=1) as ttp,
                tc.tile_pool(name="w1s", bufs=6) as w1s,
                tc.tile_pool(name="smal", bufs=12) as smal,
                tc.tile_pool(name="stg", bufs=3) as stg,
                tc.tile_pool(name="abp", bufs=3) as abp,
                tc.tile_pool(name="psab", bufs=1, space="PSUM") as psab,
                tc.tile_pool(name="pst", bufs=2, space="PSUM") as pst,
                tc.tile_pool(name="psq", bufs=2, space="PSUM") as psq,
            ):
                # resident QKV weight: [dm_part, kt, 1152]
                w1q_sb = w1qp.tile([128, KT, FQKV], BF16)
                nc.sync.dma_start(w1q_sb, w1q.rearrange("(kt p) f -> p kt f", p=128))

                for sc in range(NSC):
                    tT = ttp.tile([128, KT, SCH], BF16, tag="tT")
                    for ss in range(8):
                        s0 = sc * SCH + ss * 128
                        xt = xp.tile([128, D], BF16, tag="x")
                        nc.sync.dma_start(xt, x[s0 : s0 + 128, :])
                        s1 = smal.tile([128, 1], F32, tag="s1")
                        nc.vector.reduce_sum(s1, xt, axis=AX.X)
                        nmu = smal.tile([128, 1], F32, tag="nmu")
                        nc.scalar.mul(nmu, s1, -1.0 / D)
                        sqs = tp.tile([128, D], BF16, tag="sq", bufs=2)
                        v2 = smal.tile([128, 1], F32, tag="v2")
                        nc.scalar.activation(sqs, xt, AF.Square, bias=nmu, scale=1.0,
                                             accum_out=v2)
                        std = smal.tile([128, 1], F32, tag="std")
                        nc.scalar.activation(std, v2, AF.Sqrt, bias=eps_sb, scale=1.0 / D)
                        rstd = smal.tile([128, 1], F32, tag="rstd")
                        nc.vector.reciprocal(rstd, std)
                        nmr = smal.tile([128, 1], F32, tag="nmr")
                        nc.vector.tensor_mul(out=nmr, in0=nmu, in1=rstd)
                        tti = tp.tile([128, D], BF16, tag="t")
                        nc.scalar.activation(tti, xt, AF.Identity, bias=nmr, scale=rstd)
                        # transpose 24 [128,128] blocks -> tT, grouped 4-per-psum
                        for kg in range(KT // 4):
                            ptt = pst.tile([128, 4, 128], BF16, tag="ptt")
                            for k4 in range(4):
                                ktd = kg * 4 + k4
                                nc.tensor.transpose(
                                    ptt[:, k4, :],
                                    tti[:, ktd * 128 : (ktd + 1) * 128], ident)
                            nc.any.tensor_copy(
                                out=tT[:, kg * 4 : kg * 4 + 4, ss * 128 : (ss + 1) * 128],
                                in_=ptt)

                    # --- QKV pass (s-major): one psum bank per j ---
                    for ss in range(8):
                        s0 = sc * SCH + ss * 128
                        sl = ss * 128
                        cos_t = stg.tile([128, HD], BF16, tag="cos")
                        nc.sync.dma_start(cos_t, cosb[s0 : s0 + 128, :])
                        sin_t = stg.tile([128, HD], BF16, tag="sin")
                        nc.sync.dma_start(sin_t, sinb[s0 : s0 + 128, :])
                        s2_ = sin_t.rearrange("p (x two) -> p x two", two=2)
                        for j in range(3):
                            pq = psq.tile([128, 512], F32, tag="pqkv")
                            for kt in range(KT):
                                nc.tensor.matmul(
                                    pq[:, :FQ],
                                    tT[:, kt, sl : sl + 128],
                                    w1q_sb[:, kt, j * FQ : (j + 1) * FQ],
                                    start=(kt == 0), stop=False)
                            nc.tensor.matmul(
                                pq[:, :FQ], ones_p,
                                c2q_sb[:, j * FQ : (j + 1) * FQ],
                                start=False, stop=True)
                            if j == 2:
                                vstg = stg.tile([128, FQ], BF16, tag="vst", bufs=2)
                                nc.scalar.copy(vstg, pq[:, :FQ])
                                nc.sync.dma_start(v_d[s0 : s0 + 128, :], vstg)
                                continue
                            wb = qwb_sb if j == 0 else kwb_sb
                            qn = stg.tile([128, FQ], BF16, tag=f"qn{j}")
                            qrr = stg.tile([128, FQ], BF16, tag=f"qr{j}")
                            tmp = stg.tile([128, FQ], BF16, tag=f"tm{j}")
                            for hh in range(HL):
                                blk = pq[:, hh * HD : (hh + 1) * HD]
                                ssq = smal.tile([128, 1], F32, tag="ssq")
                                sq2 = stg.tile([128, HD], F32, tag="sq2")
                                nc.scalar.activation(sq2, blk, AF.Square, accum_out=ssq)
                                sstd = smal.tile([128, 1], F32, tag="sstd")
                                nc.scalar.activation(sstd, ssq, AF.Sqrt,
                                                     bias=eps_sb, scale=1.0 / HD)
                                rst = smal.tile([128, 1], F32, tag="rst")
                                nc.vector.reciprocal(rst, sstd)
                                qnb = qn[:, hh * HD : (hh + 1) * HD]
                                nc.vector.scalar_tensor_tensor(
                                    qnb, blk, rst, wb, ALU.mult, ALU.mult)
                                q3 = qnb.rearrange("p (x two) -> p x two", two=2)
                                t3 = tmp[:, hh * HD : (hh + 1) * HD].rearrange(
                                    "p (x two) -> p x two", two=2)
                                nc.vector.tensor_mul(out=t3[:, :, 0], in0=q3[:, :, 1],
                                                     in1=s2_[:, :, 0])
                                nc.vector.tensor_mul(out=t3[:, :, 1], in0=q3[:, :, 0],
                                                     in1=s2_[:, :, 1])
                                nc.vector.tensor_mul(
                                    out=qrr[:, hh * HD : (hh + 1) * HD],
                                    in0=qnb, in1=cos_t)
                            nc.vector.tensor_add(out=qrr, in0=qrr, in1=tmp)
                            # transpose to d-major and store
                            ptq = pst.tile([128, 4, 128], BF16, tag="ptt")
                            for hh in range(HL):
                                nc.tensor.transpose(
                                    ptq[:, hh, :],
                                    qrr[:, hh * HD : (hh + 1) * HD], ident)
                            qtr = stg.tile([128, HL, 128], BF16, tag=f"qtr{j}")
                            nc.any.tensor_copy(out=qtr, in_=ptq[:, :HL, :])
                            dstT = qT_d if j == 0 else kT_d
                            nc.sync.dma_start(
                                dstT.rearrange("(t p) s -> p t s", p=128)[
                                    :, :, s0 : s0 + 128],
                                qtr)

                    # --- a/b (f-major) + SwiGLU ---
                    for fb in range(FMT):
                        pa = psab.tile([128, 2, 2, 512], F32, tag="pab")
                        for kt in range(KT):
                            wt = w1s.tile([128, 256], BF16, tag="w1ab")
                            nc.sync.dma_start(
                                wt, w1ab[kt * 128 : (kt + 1) * 128,
                                         fb * 256 : (fb + 1) * 256])
                            for f2 in range(2):
                                for sh in range(2):
                                    nc.tensor.matmul(
                                        pa[:, f2, sh, :],
                                        wt[:, f2 * 128 : (f2 + 1) * 128],
                                        tT[:, kt, sh * 512 : (sh + 1) * 512],
                                        start=(kt == 0), stop=(kt == KT - 1))
                        a_sb = abp.tile([128, 2, 512], BF16, tag="asb")
                        nc.scalar.activation(a_sb, pa[:, 0], AF.Silu,
                                             bias=c2ab_sb[:, 2 * fb : 2 * fb + 1])
                        m_sb = abp.tile([128, 2, 512], BF16, tag="msb")
                        nc.vector.scalar_tensor_tensor(
                            m_sb, pa[:, 1], c2ab_sb[:, 2 * fb + 1 : 2 * fb + 2],
                            a_sb, ALU.add, ALU.mult)
                        nc.sync.dma_start(
                            m_f[fb * 128 : (fb + 1) * 128,
                                sc * SCH : (sc + 1) * SCH],
                            m_sb.rearrange("p a b -> p (a b)"))

            # ---------------- Phase A+O: attention + output proj + RS ---------
            with (
                tc.tile_pool(name="wo", bufs=1) as wo,
                tc.tile_pool(name="attk", bufs=1) as attk,
                tc.tile_pool(name="attv", bufs=1) as attv,
                tc.tile_pool(name="qtp", bufs=2) as qtp,
                tc.tile_pool(name="mop", bufs=2) as mop,
                tc.tile_pool(name="ptp", bufs=4) as ptp,
                tc.tile_pool(name="atts", bufs=2) as atts,
                tc.tile_pool(name="pop", bufs=3) as pop,
                tc.tile_pool(name="psS", bufs=2, space="PSUM") as psS,
                tc.tile_pool(name="psD", bufs=1, space="PSUM") as psD,
                tc.tile_pool(name="psV", bufs=1, space="PSUM") as psV,
                tc.tile_pool(name="psO", bufs=2, space="PSUM") as psO,
            ):
                wo_sb = wo.tile([128, WOT, D], BF16)
                nc.sync.dma_start(wo_sb, wout.rearrange("(kt p) n -> p kt n", p=128))
                kT_sb = attk.tile([128, HL, S], BF16)
                nc.sync.dma_start(kT_sb, kT_d.rearrange("(t p) s -> p t s", p=128))
                v_sb = attv.tile([128, NKT, FQ], BF16)
                nc.sync.dma_start(v_sb, v_d.rearrange("(t p) f -> p t f", p=128))

                for qc in range(NQC):
                    q0 = qc * QC
                    ri, rc = divmod(qc, SUBRS)
                    qt = qtp.tile([128, HL, QC], BF16, tag="qt")
                    nc.sync.dma_start(
                        qt, qT_d.rearrange("(t p) s -> p t s", p=128)[
                            :, :, q0 : q0 + QC])
                    mt = mop.tile([128, FMT, QC], BF16, tag="mt")
                    nc.sync.dma_start(
                        mt, m_f.rearrange("(t p) s -> p t s", p=128)[
                            :, :, q0 : q0 + QC])
                    aos = []
                    for h in range(HL):
                        pden = psD.tile([128, QC], F32, tag="pden")
                        pacc = psV.tile([128, QC], F32, tag="pacc")
                        pend = []

                        def drain(nc=nc, pden=pden, pacc=pacc, h=h):
                            pt, k2 = pend.pop(0)
                            for kk in range(2):
                                ki = k2 * 2 + kk
                                nc.tensor.matmul(
                                    pden, ones128, pt[:, kk, :],
                                    start=(ki == 0), stop=(ki == NKT - 1))
                                nc.tensor.matmul(
                                    pacc, v_sb[:, ki, h * HD : (h + 1) * HD],
                                    pt[:, kk, :],
                                    start=(ki == 0), stop=(ki == NKT - 1))

                        for k2 in range(NKT // 2):
                            pss = psS.tile([128, 2, QC], F32, tag="pss")
                            for kk in range(2):
                                ki = k2 * 2 + kk
                                nc.tensor.matmul(
                                    pss[:, kk, :],
                                    kT_sb[:, h, ki * 128 : (ki + 1) * 128],
                                    qt[:, h, :], start=True, stop=True)
                            if len(pend) == 2:
                                drain()
                            pt = ptp.tile([128, 2, QC], BF16, tag="pt")
                            nc.scalar.activation(pt, pss, AF.Exp)
                            pend.append((pt, k2))
                        while pend:
                            drain()
                        invd = atts.tile([128, QC], F32, tag="invd")
                        nc.vector.reciprocal(invd, pden)
                        ao = atts.tile([128, QC], BF16, tag=f"ao{h}")
                        nc.vector.tensor_mul(out=ao, in0=pacc, in1=invd)
                        aos.append(ao)
                    # output projection partial: [D, QC] = woutT @ [attn; mlp]
                    for dt in range(KT):
                        po = psO.tile([128, QC], F32, tag="po")
                        for t in range(WOT):
                            rhs = aos[t] if t < HL else mt[:, t - HL, :]
                            nc.tensor.matmul(
                                po, wo_sb[:, t, dt * 128 : (dt + 1) * 128], rhs,
                                start=(t == 0), stop=(t == WOT - 1))
                        pout = pop.tile([128, QC], BF16, tag="pout")
                        if dt % 2 == 0:
                            nc.scalar.copy(pout, po)
                        else:
                            nc.vector.tensor_copy(out=pout, in_=po)
                        nc.sync.dma_start(
                            pall[ri][dt * 128 : (dt + 1) * 128,
                                     rc * QC : (rc + 1) * QC],
                            pout)
                    if rc == SUBRS - 1:
                        nc.gpsimd.collective_compute(
                            "ReduceScatter", ALU.add, replica_groups=rg,
                            ins=[pall[ri].opt()], outs=[rs_out[ri].opt()])

            # ---------------- Epilogue: residual add on RS output -------------
            with tc.tile_pool(name="eop", bufs=2) as eop:
                for ri in range(NRS):
                    c0 = ri * RSW
                    rsb = eop.tile([128, HL, RSW], BF16, tag="rsb")
                    nc.sync.dma_start(
                        rsb, rs_out[ri].rearrange("(t p) s -> p t s", p=128))
                    rt = eop.tile([128, HL, RSW], F32, tag="rt")
                    nc.sync.dma_start(
                        rt, resT.rearrange("(t p) s -> p t s", p=128)[
                            :, :, c0 : c0 + RSW])
                    ot = eop.tile([128, HL, RSW], F32, tag="ot")
                    nc.vector.tensor_add(out=ot, in0=rt, in1=rsb)
                    nc.sync.dma_start(
                        out_t.rearrange("(t p) s -> p t s", p=128)[
                            :, :, c0 : c0 + RSW],
                        ot)

    nc.finalize()
    return nc


def _prep(inputs):
    hs = np.asarray(inputs["hidden_states"], np.float32).reshape(S, D)
    temb = np.asarray(inputs["temb_mod"], np.float32).reshape(3 * D)
    shift, scale, gate = temb[:D], temb[D : 2 * D], temb[2 * D :]
    cos = np.asarray(inputs["rotary_cos"], np.float32)
    sin = np.asarray(inputs["rotary_sin"], np.float32)
    w1 = np.asarray(inputs["w_qkv_mlp"], np.float32)
    wa = np.asarray(inputs["w_out_attn"], np.float32)
    wm = np.asarray(inputs["w_out_mlp"], np.float32)
    nqw = np.asarray(inputs["norm_q_w"], np.float32)
    nkw = np.asarray(inputs["norm_k_w"], np.float32)

    sgn = np.ones(HD, np.float32)
    sgn[0::2] = -1.0
    xb = _to_bf16(hs)
    cosb = _to_bf16(cos)
    sinb = _to_bf16(sin * sgn)
    alpha = float(HD) ** -0.25
    qwb = np.tile((nqw * alpha)[None, :], (128, 1)).astype(np.float32)
    kwb = np.tile((nkw * alpha)[None, :], (128, 1)).astype(np.float32)
    onep = (1.0 + scale)[:, None]

    in_maps = []
    for c in range(NCORES):
        q0, k0, v0 = c * FQ, D + c * FQ, 2 * D + c * FQ
        a0, b0 = 3 * D + c * FM, 3 * D + MLPH + c * FM
        w1qkv = np.concatenate(
            [w1[:, q0 : q0 + FQ], w1[:, k0 : k0 + FQ], w1[:, v0 : v0 + FQ]], axis=1)
        a_c = w1[:, a0 : a0 + FM].reshape(D, FMT, 128)
        b_c = w1[:, b0 : b0 + FM].reshape(D, FMT, 128)
        w1ab_c = np.stack([a_c, b_c], axis=2).reshape(D, FAB)
        c2q_c = (shift @ w1qkv)[None, :]
        c2ab_c = (shift @ w1ab_c).reshape(2 * FMT, 128).T
        n0 = c * NCOL
        wout_c = np.concatenate(
            [wa[c * FQ : (c + 1) * FQ, :], wm[c * FM : (c + 1) * FM, :]],
            axis=0) * gate[None, :]
        in_maps.append(dict(
            x=xb,
            w1qkv=_to_bf16(w1qkv * onep),
            w1ab=_to_bf16(w1ab_c * onep),
            c2q=_to_bf16(c2q_c),
            c2ab=np.ascontiguousarray(c2ab_c, np.float32),
            cosb=cosb, sinb=sinb, qwb=qwb, kwb=kwb,
            wout=_to_bf16(wout_c),
            resT=np.ascontiguousarray(hs[:, n0 : n0 + NCOL].T),
        ))
    return in_maps


def kernel(**inputs):
    global LAST_RESULT
    nc = _build()
    in_maps = _prep(inputs)
    r = run_bass_kernel_spmd(nc, in_maps, core_ids=list(range(NCORES)))
    LAST_RESULT = r
    full = np.concatenate([m["out"].T for m in r.results], axis=1)
    return full.reshape(1, S, D).astype(np.float32)
